# revision 45
# baseline (speedup 1.0000x reference)
"""v20: fp8-DoubleRow / fp16 rewrite of the LGeM self-attention kernel.
443us (fp32r baseline) -> 250us, rel err 1.62e-2 (gate 2e-2).

Cost-model facts this design is built around (the model is the metric):
  - fp8(e4m3) matmuls with MatmulPerfMode.DoubleRow cost 0.5 cycles per
    OUTPUT column and contract 2 k-subtiles per instruction -> 4x the
    fp32r/fp16 rate for a fixed contraction depth.
  - bf16/fp16 matmuls cost the same (1.0 cyc/col), so all non-fp8 math
    runs in fp16 (8x less rounding noise than bf16 for free).
  - ACT exp costs 0.833ns/elem + ~190ns/inst regardless of dtype.
  - DVE fp16 elementwise ops hit the 2x perf mode (~330ns per [128,512]).
  - Every DMA transfer serializes on one shared device (~360GB/s) in
    issue order, and each DMA also costs ~625ns on a serial issue device
    -> few, large, criticality-ordered transfers on a single queue.
  - PSUM is 8 banks; one unified pool (w0,w1 = [128,2,512]; o2p0,o2p1,
    sp,fp = [128,512]) serves every phase with zero scope barriers.

Structure (per core: one batch element x 4 heads; host sums the 4
tensor-parallel partials per batch):
  A1  v-proj chain 1 (x_hi @ Wv_hi, fp8 DR) -> v_sb fp16 (holds 64*v).
  A2  q/k-proj (fp8 DR, weights pre-scaled 64x) -> DVE rope-mul by
      (cos+sin) factors (with 1/sqrt(t) and 1/64 folded in) -> fp16
      qT/kT. The first half of A2 interleaves into A1's DMA-paced
      rounds (x8hi ships in column halves to enable this).
  A3  v-proj correction chains (x_lo@Wv_hi + x_hi@Wv_lo) compensate the
      fp8 quantization of x and Wv; the last 4 tk tiles defer into phase
      B's warm-up slots (PE-idle while the exp pipeline fills).
  B   per (tq, head) slot: fp16 scores -> one exp per 2-bank PSUM group
      (fp16 et) -> fp16 attn@v lagging the exp stream by 4 groups (hides
      all cross-engine latency) -> denominator via DVE/Pool fp16 pair
      tree + one ones[128,128]@acc matmul (replaces 16 PE matmuls).
      Head finishers (denominator matmul, reciprocal, normalize+split to
      fp8 hi/lo) and out-proj groups (3-chain compensated fp8 DR over
      (d, head-pair)) drip through the next slot's stream so PE never
      drains; finished 512-row blocks leave as single batched DMAs.

Numerics: fp8 quantization residuals are compensated where errors pass
straight through (v, out-proj), and tolerated where softmax attenuates
them (x/W fp8 noise lands on logits ~N(0,0.2^2) as ~1.7% attention
weight noise). Scores/et/v/out2 run in fp16. Measured 1.62e-2 absmax
rel err on the reference inputs (deterministic).
"""

import sys

sys.path.insert(0, "/opt/trn_rl_repo")

import math

import numpy as np
import ml_dtypes

import concourse.bass as bass
import concourse.mybir as mybir
import concourse.tile as tile
from concourse import bacc, bass_utils

F32 = mybir.dt.float32
FP16 = mybir.dt.float16
FP8 = mybir.dt.float8e4
NP_FP8 = ml_dtypes.float8_e4m3
Copy = mybir.ActivationFunctionType.Copy
Exp = mybir.ActivationFunctionType.Exp
DR = mybir.MatmulPerfMode.DoubleRow

HIDDEN = 2048
HEADS = 16
HEAD_DIM = 128
SEQ = 2048
BATCH = 2
N_CORES = 8
MP = 4
HG = HEADS // MP  # 4 heads per core
THETA = 10000.0

WSCALE = 64.0  # host premultiplier on Wq/Wk/Wv/Wo before fp8 cast
O2SCALE = 32.0  # fp8 scale on the normalized attention output


def build_attention_nc(use_mask=False):
    T, C, D, DG = SEQ, HIDDEN, HEAD_DIM, HG * HEAD_DIM
    CCH = C // 128  # 16 contraction subtiles
    CCP = CCH // 2  # 8 DoubleRow pairs
    NTK = T // 128  # 16
    NTQ = T // 512  # 4 tq blocks
    NG = NTK // 2  # 8 exp groups per (h, tq)

    nc = bacc.Bacc("TRN2", target_bir_lowering=False, debug=False)

    x8hi = nc.dram_tensor("x8hi", [C, T], FP8, kind="ExternalInput").ap()
    x8lo = nc.dram_tensor("x8lo", [C, T], FP8, kind="ExternalInput").ap()
    wv8hi = nc.dram_tensor("wv8hi", [C, DG], FP8, kind="ExternalInput").ap()
    wv8lo = nc.dram_tensor("wv8lo", [C, DG], FP8, kind="ExternalInput").ap()
    w8q = nc.dram_tensor("w8q", [C, DG], FP8, kind="ExternalInput").ap()
    w8k = nc.dram_tensor("w8k", [C, DG], FP8, kind="ExternalInput").ap()
    wo8hi = nc.dram_tensor("wo8hi", [HG, D, C], FP8, kind="ExternalInput").ap()
    wo8lo = nc.dram_tensor("wo8lo", [HG, D, C], FP8, kind="ExternalInput").ap()
    cfq = nc.dram_tensor("cfq", [D, T], FP16, kind="ExternalInput").ap()
    cfk = nc.dram_tensor("cfk", [D, T], FP16, kind="ExternalInput").ap()
    if use_mask:
        maskT = nc.dram_tensor("maskT", [T, T], F32, kind="ExternalInput").ap()
    out = nc.dram_tensor("out", [T, C], FP16, kind="ExternalOutput").ap()

    with tile.TileContext(nc) as tc:
        with (
            tc.tile_pool(name="res", bufs=1) as res,
            tc.tile_pool(name="etp", bufs=1) as etp,
            tc.tile_pool(name="accp", bufs=2) as accp,
            tc.tile_pool(name="o2sb", bufs=2) as o2sb,
            tc.tile_pool(name="rtp", bufs=2) as rtp,
            tc.tile_pool(name="otp", bufs=2) as otp,
            tc.tile_pool(name="mkp", bufs=2) as mkp,
        ):
            wo8hi_s = res.tile([128, HG, C], FP8, tag="wo8hi_s")
            wo8lo_s = res.tile([128, HG, C], FP8, tag="wo8lo_s")
            cfq_s = res.tile([128, T], FP16, tag="cfq_s")
            cfk_s = res.tile([128, T], FP16, tag="cfk_s")
            v_sb = res.tile([128, NTK, DG], FP16, tag="v_sb")
            qT16 = res.tile([128, HG, T], FP16, tag="qT16")
            kT16 = res.tile([128, HG, T], FP16, tag="kT16")
            ones = res.tile([128, 128], FP16, tag="ones")

            # ones = WSCALE/O2SCALE: v_sb holds 64*v and the normalize
            # multiply should emit 32*out2 (the fp8 out-proj scale), so the
            # denominator matmul pre-scales by 64/32 = 2.
            nc.vector.memset(ones[:], WSCALE / O2SCALE)

            # ---- phase A (x/w staging SBUF is scoped so phase B can hold
            # a full slot of et tiles) ----
            _ps_cm = tc.tile_pool(name="ps", bufs=1, space="PSUM")
            ps = _ps_cm.__enter__()

            def ps_pair(tag):
                return ps.tile([128, 2, 512], F32, tag=tag, name=tag)

            def ps_one(tag):
                return ps.tile([128, 512], F32, tag=tag, name=tag)

            _xw_cm = tc.tile_pool(name="xw", bufs=1)
            xw = _xw_cm.__enter__()
            _w8_cm = tc.tile_pool(name="w8p", bufs=1)
            w8p = _w8_cm.__enter__()
            if True:
                x8hi_s = xw.tile([128, CCH, T], FP8, tag="x8hi_s")
                x8lo_s = xw.tile([128, CCH, T], FP8, tag="x8lo_s")
                wv8hi_s = xw.tile([128, CCH, DG], FP8, tag="wv8hi_s")
                wv8lo_s = xw.tile([128, CCH, DG], FP8, tag="wv8lo_s")
                w8q_s = w8p.tile([128, CCH, DG], FP8, tag="w8q_s")
                w8k_s = w8p.tile([128, CCH, DG], FP8, tag="w8k_s")

                # Input loads: ALL on one queue (SP): transfers from
                # different queues round-robin on the shared DMA device, so
                # a single queue is the only way to enforce global
                # dependency-criticality order. x8hi ships in column halves
                # so q/k-projection of the first two tq blocks (and the
                # first v rounds) can start before the full tensor lands.
                wvh_r = wv8hi.rearrange("(a p) b -> p a b", p=128)
                wvl_r = wv8lo.rearrange("(a p) b -> p a b", p=128)
                xhi_r = x8hi.rearrange("(a p) t -> p a t", p=128)
                xlo_r = x8lo.rearrange("(a p) t -> p a t", p=128)
                H2 = T // 2
                nc.sync.dma_start(wv8hi_s[:, 0:4, :], wvh_r[:, 0:4, :])
                nc.sync.dma_start(x8hi_s[:, 0:4, 0:H2], xhi_r[:, 0:4, 0:H2])
                nc.sync.dma_start(x8hi_s[:, 4:8, 0:H2], xhi_r[:, 4:8, 0:H2])
                nc.sync.dma_start(wv8hi_s[:, 4:16, :], wvh_r[:, 4:16, :])
                nc.sync.dma_start(x8hi_s[:, 8:16, 0:H2], xhi_r[:, 8:16, 0:H2])
                nc.sync.dma_start(
                    w8q_s[:, :, :], w8q.rearrange("(a p) b -> p a b", p=128)
                )
                nc.sync.dma_start(
                    w8k_s[:, :, :], w8k.rearrange("(a p) b -> p a b", p=128)
                )
                nc.sync.dma_start(x8hi_s[:, 0:8, H2:T], xhi_r[:, 0:8, H2:T])
                nc.sync.dma_start(x8hi_s[:, 8:16, H2:T], xhi_r[:, 8:16, H2:T])
                nc.sync.dma_start(cfq_s[:D, :], cfq)
                nc.sync.dma_start(cfk_s[:D, :], cfk)
                nc.sync.dma_start(wv8lo_s[:, :, :], wvl_r)
                nc.sync.dma_start(x8lo_s[:, 0:8, :], xlo_r[:, 0:8, :])
                nc.sync.dma_start(x8lo_s[:, 8:16, :], xlo_r[:, 8:16, :])
                nc.sync.dma_start(
                    wo8hi_s[:D, :, :], wo8hi.rearrange("h p c -> p h c")
                )
                nc.sync.dma_start(
                    wo8lo_s[:D, :, :], wo8lo.rearrange("h p c -> p h c")
                )

                # A1/A3 rounds and A2's qk tiles all rotate through one
                # 8-bank tag space -- no PSUM scope transitions in phase A.
                def alloc_accs(n):
                    accs = []
                    if n > 0:
                        w = ps_pair("w0")
                        accs += [w[:, 0, :], w[:, 1, :]]
                    if n > 2:
                        w = ps_pair("w1")
                        accs += [w[:, 0, :], w[:, 1, :]]
                    for tag in ("o2p0", "o2p1", "sp", "fp")[: max(0, n - 4)]:
                        accs.append(ps_one(tag)[:, :])
                    return accs[:n]

                def alloc_sing(n, off=0):
                    return [
                        ps_one(tag)[:, :]
                        for tag in ("o2p0", "o2p1", "sp", "fp")[off : off + n]
                    ]

                def vchain(round_tks, chains, vop, accs):
                    pvs = accs
                    n = len(chains) * CCP
                    k = 0
                    for ccp in range(CCP):
                        for x_s, w_s in chains:
                            for i, tk in enumerate(round_tks):
                                nc.tensor.matmul(
                                    pvs[i][:],
                                    x_s[:, 2 * ccp : 2 * ccp + 2, tk * 128 : (tk + 1) * 128],
                                    w_s[:, 2 * ccp : 2 * ccp + 2, :],
                                    start=(k == 0),
                                    stop=(k == n - 1),
                                    perf_mode=DR,
                                )
                            k += 1
                    for i, tk in enumerate(round_tks):
                        vop(tk, pvs[i])

                # A1: v-proj chain 1 (x_hi @ Wv_hi) -> v_sb (holds 64*v);
                # PSUM->SBUF copies alternate DVE/ACT
                def a1_copy(tk, pv):
                    if tk % 2 == 0:
                        nc.vector.tensor_copy(v_sb[:, tk, :], pv[:])
                    else:
                        nc.scalar.activation(v_sb[:, tk, :], pv[:], Copy)

                a3_add = lambda tk, pv: nc.vector.tensor_add(
                    v_sb[:, tk, :], v_sb[:, tk, :], pv[:]
                )

                # q/k tile generator: one projection output tile at a time,
                # using a caller-supplied PSUM slot
                def qk_tile(h, wi, tq, qp):
                    w_s, cf_s, dst = (
                        (w8q_s, cfq_s, qT16) if wi == 0 else (w8k_s, cfk_s, kT16)
                    )
                    for ccp in range(CCP):
                        nc.tensor.matmul(
                            qp[:],
                            w_s[:, 2 * ccp : 2 * ccp + 2, h * D : (h + 1) * D],
                            x8hi_s[:, 2 * ccp : 2 * ccp + 2, tq * 512 : (tq + 1) * 512],
                            start=(ccp == 0),
                            stop=(ccp == CCP - 1),
                            perf_mode=DR,
                        )
                    nc.vector.tensor_mul(
                        dst[:D, h, tq * 512 : (tq + 1) * 512],
                        qp[:D, :],
                        cf_s[:D, tq * 512 : (tq + 1) * 512],
                    )

                qk_left = [
                    (h, wi, tq) for h in range(HG) for wi in (0, 1) for tq in (0, 1)
                ]
                qk_right = [
                    (h, wi, tq) for h in range(HG) for wi in (0, 1) for tq in (2, 3)
                ]
                spfp = [0]

                def emit_qk(units, n):
                    # constrained 2-bank rotation (sp/fp) for tiles that
                    # interleave with open v rounds
                    for _ in range(n):
                        if not units:
                            return
                        h, wi, tq = units.pop(0)
                        qp = ps_one("sp" if spfp[0] % 2 == 0 else "fp")
                        spfp[0] += 1
                        qk_tile(h, wi, tq, qp)

                qk_accs = [None]
                qk_i = [0]

                def emit_qk_full(units, n):
                    # full 8-slot rotation once all v banks are free
                    for _ in range(n):
                        if not units:
                            return
                        h, wi, tq = units.pop(0)
                        if qk_i[0] % 8 == 0:
                            qk_accs[0] = alloc_accs(8)
                        qp = qk_accs[0][qk_i[0] % 8]
                        qk_i[0] += 1
                        qk_tile(h, wi, tq, qp)

                # r0 (tk0-5): paced by the x8hi left half
                vchain(list(range(6)), [(x8hi_s, wv8hi_s)], a1_copy, alloc_accs(6))
                # r1 (tk6-11): tk8+ paced by the right half; interleave the
                # left-half q/k tiles (ready once w8q/w8k land) on sp/fp
                pvs = alloc_accs(6)
                for ccp in range(CCP):
                    for i, tk in enumerate(range(6, 12)):
                        nc.tensor.matmul(
                            pvs[i][:],
                            x8hi_s[:, 2 * ccp : 2 * ccp + 2, tk * 128 : (tk + 1) * 128],
                            wv8hi_s[:, 2 * ccp : 2 * ccp + 2, :],
                            start=(ccp == 0),
                            stop=(ccp == CCP - 1),
                            perf_mode=DR,
                        )
                    emit_qk(qk_left, 2)
                for i, tk in enumerate(range(6, 12)):
                    a1_copy(tk, pvs[i])
                # r2 (tk12-15) + remaining left-half q/k tiles
                pvs = alloc_accs(4)
                for ccp in range(CCP):
                    for i, tk in enumerate(range(12, 16)):
                        nc.tensor.matmul(
                            pvs[i][:],
                            x8hi_s[:, 2 * ccp : 2 * ccp + 2, tk * 128 : (tk + 1) * 128],
                            wv8hi_s[:, 2 * ccp : 2 * ccp + 2, :],
                            start=(ccp == 0),
                            stop=(ccp == CCP - 1),
                            perf_mode=DR,
                        )
                    emit_qk(qk_left, 1)
                for i, tk in enumerate(range(12, 16)):
                    a1_copy(tk, pvs[i])
                emit_qk_full(qk_left, 99)
                emit_qk_full(qk_right, 99)
                _w8_cm.__exit__(None, None, None)

                # A3: v-proj correction chains (x_lo@Wv_hi + x_hi@Wv_lo);
                # later rounds avoid w0/w1 so phase B's score banks free early
                a3_chains = [(x8lo_s, wv8hi_s), (x8hi_s, wv8lo_s)]
                vchain(list(range(0, 8)), a3_chains, a3_add, alloc_accs(8))
                vchain(list(range(8, 12)), a3_chains, a3_add, alloc_accs(4))
                # tk12-15 are deferred: their 64 matmuls drip into phase B's
                # first two slots (which are PE-idle during pipeline warm-up)
                # on the single-bank tags, which phase B needs only at slot 2+
                a3_accs = alloc_sing(2) + alloc_sing(2, off=2)
                a3_tail = []
                for i, tk in enumerate(range(12, 16)):
                    for ccp in range(CCP):
                        for ci, (x_s, w_s) in enumerate(a3_chains):
                            a3_tail.append(
                                (a3_accs[i], x_s, w_s, ccp, tk,
                                 ccp == 0 and ci == 0,
                                 ccp == CCP - 1 and ci == 1)
                            )

                def drip_a3(n):
                    for _ in range(n):
                        if not a3_tail:
                            return
                        pv, x_s, w_s, ccp, tk, st, sp_ = a3_tail.pop(0)
                        nc.tensor.matmul(
                            pv[:],
                            x_s[:, 2 * ccp : 2 * ccp + 2, tk * 128 : (tk + 1) * 128],
                            w_s[:, 2 * ccp : 2 * ccp + 2, :],
                            start=st,
                            stop=sp_,
                            perf_mode=DR,
                        )
                        if sp_:
                            a3_add(tk, pv)

            # ---- phase B: slot-pipelined attention + out-projection ----
            # Slot k = (tq, h). PE emits scores(k) interleaved with
            # attn@v(k-1) -- a FULL slot of lag, so attn@v never waits on
            # ACT's exp stream. Finisher(k-1) (denominator matmul, recip,
            # normalize) and out-proj drips fill the remaining PE slack.
            if True:
                og_queue = []
                og_i = [0]
                ot4 = [None]
                fin_queue = []
                TAGROT = ["fp", "sp"]

                def emit_og(tags=("fp", "sp")):
                    tag = tags[og_i[0] % len(tags)]
                    og_i[0] += 1
                    qt, oc, o2t, tq = og_queue.pop(0)
                    fp = (
                        ps_one(tag) if tag in ("fp", "sp", "o2p0", "o2p1")
                        else ps_pair(tag[:2])[:, int(tag[3]), :]
                    )
                    o2h, o2l = o2t
                    chains = ((o2h, wo8hi_s), (o2l, wo8hi_s), (o2h, wo8lo_s))
                    n = len(chains) * 2
                    ci = 0
                    for lhs, rhs in chains:
                        for hp in (0, 2):
                            nc.tensor.matmul(
                                fp[:],
                                lhs[:D, hp : hp + 2, qt * 128 : (qt + 1) * 128],
                                rhs[:D, hp : hp + 2, oc * 512 : (oc + 1) * 512],
                                start=(ci == 0),
                                stop=(ci == n - 1),
                                perf_mode=DR,
                            )
                            ci += 1
                    if oc == 0:
                        ot4[0] = otp.tile([128, 4, 512], FP16, tag="ot", name="ot")
                    ot = ot4[0]
                    if og_i[0] % 2 == 0:
                        nc.scalar.activation(
                            ot[:, oc, :],
                            fp[:],
                            Copy,
                            scale=float(1.0 / (WSCALE * O2SCALE)),
                        )
                    else:
                        nc.vector.tensor_scalar_mul(
                            ot[:, oc, :], fp[:], 1.0 / (WSCALE * O2SCALE)
                        )
                    if oc == 3:
                        nc.sync.dma_start(
                            out[tq * 512 + qt * 128 : tq * 512 + (qt + 1) * 128, :],
                            ot[:],
                        )

                def flush_fin(tag="sp"):
                    # out-proj groups for a tq block enter the queue HERE,
                    # once its last head is normalized -- dripping them any
                    # earlier would read o2t[h3] before it is written.
                    acc, o2p, o2t, h, tq = fin_queue.pop(0)
                    spp = ps_one(tag)
                    nc.tensor.matmul(spp[:], ones[:], acc[:], start=True, stop=True)
                    rt = rtp.tile([128, 512], F32, tag="rt", bufs=1)
                    nc.vector.reciprocal(rt[:], spp[:])
                    m32 = rtp.tile([128, 512], FP16, tag="m32", bufs=2)
                    nc.vector.tensor_mul(m32[:D, :], o2p[:D, :], rt[:D, :])
                    o2h, o2l = o2t
                    nc.vector.tensor_copy(o2h[:D, h, :], m32[:D, :])
                    nc.vector.tensor_tensor(o2l[:D, h, :], m32[:D, :], o2h[:D, h, :], mybir.AluOpType.subtract)
                    if h == HG - 1:
                        for qt in range(4):
                            for oc in range(4):
                                og_queue.append((qt, oc, o2t, tq))

                slots = [(tq, h) for tq in range(NTQ) for h in range(HG)]
                pend = []  # (ctx, g) attn@v work lagging 4 exp groups

                def emit_av(ctx, g):
                    ets, o2p, o2t, h, acc, _tq = ctx
                    et = ets[g]
                    for j in range(2):
                        tk = 2 * g + j
                        nc.tensor.matmul(
                            o2p[:],
                            v_sb[:, tk, h * D : (h + 1) * D],
                            et[:, j, :],
                            start=(tk == 0),
                            stop=(tk == NTK - 1),
                        )
                    eng = nc.vector if g % 2 == 0 else nc.gpsimd
                    eng.tensor_add(et[:, 0, :], et[:, 0, :], et[:, 1, :])
                    if g == 1:
                        nc.vector.tensor_add(acc[:], ets[0][:, 0, :], ets[1][:, 0, :])
                    elif g > 1:
                        nc.vector.tensor_add(acc[:], acc[:], et[:, 0, :])

                for k, (tq, h) in enumerate(slots):
                    if h == 0:
                        o2t = (
                            o2sb.tile([128, HG, 512], FP8, tag="o2h", name="o2h"),
                            o2sb.tile([128, HG, 512], FP8, tag="o2l", name="o2l"),
                        )
                    o2p = ps_one(f"o2p{k % 2}")
                    acc = accp.tile([128, 512], FP16, tag="acc")
                    ets = []
                    ctx = (ets, o2p, o2t, h, acc, tq)
                    for g in range(NG):
                        sc = ps_pair(f"w{g % 2}")
                        for j in range(2):
                            tk = 2 * g + j
                            nc.tensor.matmul(
                                sc[:, j, :],
                                kT16[:D, h, tk * 128 : (tk + 1) * 128],
                                qT16[:D, h, tq * 512 : (tq + 1) * 512],
                                start=True,
                                stop=True,
                            )
                        if use_mask:
                            mt = mkp.tile([128, 2, 512], F32, tag="mt")
                            for j in range(2):
                                nc.gpsimd.dma_start(
                                    mt[:, j, :],
                                    maskT[
                                        (2 * g + j) * 128 : (2 * g + j + 1) * 128,
                                        tq * 512 : (tq + 1) * 512,
                                    ],
                                )
                            nc.vector.tensor_add(sc[:, :, :], sc[:, :, :], mt[:])
                        et = etp.tile(
                            [128, 2, 512], FP16, tag=f"et{(k * NG + g) % 8}"
                        )
                        nc.scalar.activation(et[:], sc[:], Exp)
                        ets.append(et)
                        drip_a3(4)
                        # attn@v lags the exp stream by 4 groups
                        pend.append((ctx, g))
                        if len(pend) > 5:
                            actx, ag = pend.pop(0)
                            emit_av(actx, ag)
                            if ag == NG - 1:
                                fin_queue.append(
                                    (actx[4], actx[1], actx[2], actx[3], actx[5])
                                )
                        if g == 4 and fin_queue:
                            flush_fin()
                        if g in (3, 5, 7) and og_queue:
                            emit_og()
                            if g == 7 and og_queue and len(og_queue) % 4 == 1:
                                emit_og()
                _xw_cm.__exit__(None, None, None)
                # drain: pending attn@v groups, remaining finishers, then the
                # final out-proj groups rotating over the idle banks
                di = 0
                while pend:
                    actx, ag = pend.pop(0)
                    emit_av(actx, ag)
                    if ag == NG - 1:
                        fin_queue.append(
                            (actx[4], actx[1], actx[2], actx[3], actx[5])
                        )
                    if di % 2 == 1 and og_queue:
                        emit_og()
                    di += 1
                while fin_queue:
                    flush_fin("sp" if len(fin_queue) % 2 == 0 else "fp")
                og_i[0] = 0
                while og_queue:
                    emit_og(
                        ("fp", "w0_0", "sp", "w1_0", "o2p0", "w0_1", "o2p1", "w1_1")
                    )

            _ps_cm.__exit__(None, None, None)

    nc.compile()
    return nc


def compute_cfacs():
    T, D = SEQ, HEAD_DIM
    freq = 1.0 / THETA ** (np.arange(0, D, 2, dtype=np.float64) / D)
    t = np.arange(T, dtype=np.float64)
    m = np.einsum("i,j->ij", t, freq)
    m = np.concatenate([m, m], axis=-1)
    cfac = (np.cos(m) + np.sin(m)).T  # [D, T]
    cfq = (cfac / math.sqrt(T) / WSCALE).astype(np.float16)
    cfk = (cfac / WSCALE).astype(np.float16)
    return cfq, cfk


_NC_CACHE = {}


def _get_nc(use_mask):
    key = bool(use_mask)
    if key not in _NC_CACHE:
        _NC_CACHE[key] = build_attention_nc(use_mask=key)
    return _NC_CACHE[key]


def _split_fp8(a):
    hi = a.astype(NP_FP8)
    lo = (a - hi.astype(np.float32)).astype(NP_FP8)
    return hi, lo


def _make_in_maps(input_ids, Wq, Wk, Wv, Wo, attention_mask=None):
    DG = HG * HEAD_DIM
    cfq, cfk = compute_cfacs()
    xhi, xlo = [], []
    for bi in range(BATCH):
        xT = np.ascontiguousarray(input_ids[bi].T)
        hi, lo = _split_fp8(xT)
        xhi.append(hi)
        xlo.append(lo)
    in_maps = []
    for core in range(N_CORES):
        bi, g = divmod(core, MP)
        sl = slice(g * DG, (g + 1) * DG)
        wvhi, wvlo = _split_fp8(np.ascontiguousarray(Wv[:, sl]) * WSCALE)
        wohi, wolo = _split_fp8(
            np.ascontiguousarray(Wo[sl, :]).reshape(HG, HEAD_DIM, HIDDEN) * WSCALE
        )
        m = {
            "x8hi": xhi[bi],
            "x8lo": xlo[bi],
            "wv8hi": wvhi,
            "wv8lo": wvlo,
            "w8q": (np.ascontiguousarray(Wq[:, sl]) * WSCALE).astype(NP_FP8),
            "w8k": (np.ascontiguousarray(Wk[:, sl]) * WSCALE).astype(NP_FP8),
            "wo8hi": wohi,
            "wo8lo": wolo,
            "cfq": cfq,
            "cfk": cfk,
        }
        if attention_mask is not None:
            m["maskT"] = np.ascontiguousarray(attention_mask[bi, 0].T).astype(
                np.float32
            )
        in_maps.append(m)
    return in_maps


def prepare_for_bench(inputs):
    input_ids = np.asarray(inputs["input_ids"], dtype=np.float32)
    Wq = np.asarray(inputs["Wq"], dtype=np.float32)
    Wk = np.asarray(inputs["Wk"], dtype=np.float32)
    Wv = np.asarray(inputs["Wv"], dtype=np.float32)
    Wo = np.asarray(inputs["Wo"], dtype=np.float32)
    return _get_nc(False), _make_in_maps(input_ids, Wq, Wk, Wv, Wo)


def kernel(input_ids, attention_mask, Wq, Wk, Wv, Wo):
    input_ids = np.asarray(input_ids, dtype=np.float32)
    attention_mask = np.asarray(attention_mask, dtype=np.float32)
    Wq = np.asarray(Wq, dtype=np.float32)
    Wk = np.asarray(Wk, dtype=np.float32)
    Wv = np.asarray(Wv, dtype=np.float32)
    Wo = np.asarray(Wo, dtype=np.float32)

    b, t, c = input_ids.shape
    assert (b, t, c) == (BATCH, SEQ, HIDDEN)

    use_mask = bool(np.any(attention_mask))
    nc = _get_nc(use_mask)
    in_maps = _make_in_maps(
        input_ids, Wq, Wk, Wv, Wo, attention_mask if use_mask else None
    )

    res = bass_utils.run_bass_kernel_spmd(nc, in_maps, core_ids=list(range(N_CORES)))

    out = np.zeros((BATCH, SEQ, HIDDEN), dtype=np.float32)
    for bi in range(BATCH):
        acc = res.results[bi * MP]["out"].astype(np.float32)
        for g in range(1, MP):
            acc = acc + res.results[bi * MP + g]["out"].astype(np.float32)
        out[bi] = acc
    return out


# revision 46
# speedup vs baseline: 1.0186x; 1.0186x over previous
"""v20: fp8-DoubleRow / fp16 rewrite of the LGeM self-attention kernel.
443us (fp32r baseline) -> 250us, rel err 1.62e-2 (gate 2e-2).

Cost-model facts this design is built around (the model is the metric):
  - fp8(e4m3) matmuls with MatmulPerfMode.DoubleRow cost 0.5 cycles per
    OUTPUT column and contract 2 k-subtiles per instruction -> 4x the
    fp32r/fp16 rate for a fixed contraction depth.
  - bf16/fp16 matmuls cost the same (1.0 cyc/col), so all non-fp8 math
    runs in fp16 (8x less rounding noise than bf16 for free).
  - ACT exp costs 0.833ns/elem + ~190ns/inst regardless of dtype.
  - DVE fp16 elementwise ops hit the 2x perf mode (~330ns per [128,512]).
  - Every DMA transfer serializes on one shared device (~360GB/s) in
    issue order, and each DMA also costs ~625ns on a serial issue device
    -> few, large, criticality-ordered transfers on a single queue.
  - PSUM is 8 banks; one unified pool (w0,w1 = [128,2,512]; o2p0,o2p1,
    sp,fp = [128,512]) serves every phase with zero scope barriers.

Structure (per core: one batch element x 4 heads; host sums the 4
tensor-parallel partials per batch):
  A1  v-proj chain 1 (x_hi @ Wv_hi, fp8 DR) -> v_sb fp16 (holds 64*v).
  A2  q/k-proj (fp8 DR, weights pre-scaled 64x) -> DVE rope-mul by
      (cos+sin) factors (with 1/sqrt(t) and 1/64 folded in) -> fp16
      qT/kT. The first half of A2 interleaves into A1's DMA-paced
      rounds (x8hi ships in column halves to enable this).
  A3  v-proj correction chains (x_lo@Wv_hi + x_hi@Wv_lo) compensate the
      fp8 quantization of x and Wv; the last 4 tk tiles defer into phase
      B's warm-up slots (PE-idle while the exp pipeline fills).
  B   per (tq, head) slot: fp16 scores -> one exp per 2-bank PSUM group
      (fp16 et) -> fp16 attn@v lagging the exp stream by 4 groups (hides
      all cross-engine latency) -> denominator via DVE/Pool fp16 pair
      tree + one ones[128,128]@acc matmul (replaces 16 PE matmuls).
      Head finishers (denominator matmul, reciprocal, normalize+split to
      fp8 hi/lo) and out-proj groups (3-chain compensated fp8 DR over
      (d, head-pair)) drip through the next slot's stream so PE never
      drains; finished 512-row blocks leave as single batched DMAs.

Numerics: fp8 quantization residuals are compensated where errors pass
straight through (v, out-proj), and tolerated where softmax attenuates
them (x/W fp8 noise lands on logits ~N(0,0.2^2) as ~1.7% attention
weight noise). Scores/et/v/out2 run in fp16. Measured 1.62e-2 absmax
rel err on the reference inputs (deterministic).
"""

import sys

sys.path.insert(0, "/opt/trn_rl_repo")

import math

import numpy as np
import ml_dtypes

import concourse.bass as bass
import concourse.mybir as mybir
import concourse.tile as tile
from concourse import bacc, bass_utils

F32 = mybir.dt.float32
FP16 = mybir.dt.float16
FP8 = mybir.dt.float8e4
NP_FP8 = ml_dtypes.float8_e4m3
Copy = mybir.ActivationFunctionType.Copy
Exp = mybir.ActivationFunctionType.Exp
DR = mybir.MatmulPerfMode.DoubleRow

HIDDEN = 2048
HEADS = 16
HEAD_DIM = 128
SEQ = 2048
BATCH = 2
N_CORES = 8
MP = 4
HG = HEADS // MP  # 4 heads per core
THETA = 10000.0

WSCALE = 64.0  # host premultiplier on Wq/Wk/Wv/Wo before fp8 cast
O2SCALE = 32.0  # fp8 scale on the normalized attention output


def build_attention_nc(use_mask=False):
    T, C, D, DG = SEQ, HIDDEN, HEAD_DIM, HG * HEAD_DIM
    CCH = C // 128  # 16 contraction subtiles
    CCP = CCH // 2  # 8 DoubleRow pairs
    NTK = T // 128  # 16
    NTQ = T // 512  # 4 tq blocks
    NG = NTK // 2  # 8 exp groups per (h, tq)

    nc = bacc.Bacc("TRN2", target_bir_lowering=False, debug=False)

    x8hi = nc.dram_tensor("x8hi", [C, T], FP8, kind="ExternalInput").ap()
    x8lo = nc.dram_tensor("x8lo", [C, T], FP8, kind="ExternalInput").ap()
    wv8hi = nc.dram_tensor("wv8hi", [C, DG], FP8, kind="ExternalInput").ap()
    wv8lo = nc.dram_tensor("wv8lo", [C, DG], FP8, kind="ExternalInput").ap()
    w8q = nc.dram_tensor("w8q", [C, DG], FP8, kind="ExternalInput").ap()
    w8k = nc.dram_tensor("w8k", [C, DG], FP8, kind="ExternalInput").ap()
    wo8hi = nc.dram_tensor("wo8hi", [HG, D, C], FP8, kind="ExternalInput").ap()
    wo8lo = nc.dram_tensor("wo8lo", [HG, D, C], FP8, kind="ExternalInput").ap()
    cfq = nc.dram_tensor("cfq", [D, T], FP16, kind="ExternalInput").ap()
    cfk = nc.dram_tensor("cfk", [D, T], FP16, kind="ExternalInput").ap()
    if use_mask:
        maskT = nc.dram_tensor("maskT", [T, T], F32, kind="ExternalInput").ap()
    out = nc.dram_tensor("out", [T, C], FP16, kind="ExternalOutput").ap()

    with tile.TileContext(nc) as tc:
        with (
            tc.tile_pool(name="res", bufs=1) as res,
            tc.tile_pool(name="etp", bufs=1) as etp,
            tc.tile_pool(name="accp", bufs=2) as accp,
            tc.tile_pool(name="o2sb", bufs=2) as o2sb,
            tc.tile_pool(name="rtp", bufs=2) as rtp,
            tc.tile_pool(name="otp", bufs=2) as otp,
            tc.tile_pool(name="mkp", bufs=2) as mkp,
        ):
            wo8hi_s = res.tile([128, HG, C], FP8, tag="wo8hi_s")
            wo8lo_s = res.tile([128, HG, C], FP8, tag="wo8lo_s")
            cfq_s = res.tile([128, T], FP16, tag="cfq_s")
            cfk_s = res.tile([128, T], FP16, tag="cfk_s")
            v_sb = res.tile([128, NTK, DG], FP16, tag="v_sb")
            qT16 = res.tile([128, HG, T], FP16, tag="qT16")
            kT16 = res.tile([128, HG, T], FP16, tag="kT16")
            ones = res.tile([128, 128], FP16, tag="ones")

            # ones = WSCALE/O2SCALE: v_sb holds 64*v and the normalize
            # multiply should emit 32*out2 (the fp8 out-proj scale), so the
            # denominator matmul pre-scales by 64/32 = 2.
            nc.vector.memset(ones[:], WSCALE / O2SCALE)

            # ---- phase A (x/w staging SBUF is scoped so phase B can hold
            # a full slot of et tiles) ----
            _ps_cm = tc.tile_pool(name="ps", bufs=1, space="PSUM")
            ps = _ps_cm.__enter__()

            def ps_pair(tag):
                return ps.tile([128, 2, 512], F32, tag=tag, name=tag)

            def ps_one(tag):
                return ps.tile([128, 512], F32, tag=tag, name=tag)

            _xw_cm = tc.tile_pool(name="xw", bufs=1)
            xw = _xw_cm.__enter__()
            _w8_cm = tc.tile_pool(name="w8p", bufs=1)
            w8p = _w8_cm.__enter__()
            if True:
                x8hi_s = xw.tile([128, CCH, T], FP8, tag="x8hi_s")
                x8lo_s = xw.tile([128, CCH, T], FP8, tag="x8lo_s")
                wv8hi_s = xw.tile([128, CCH, DG], FP8, tag="wv8hi_s")
                wv8lo_s = xw.tile([128, CCH, DG], FP8, tag="wv8lo_s")
                w8q_s = w8p.tile([128, CCH, DG], FP8, tag="w8q_s")
                w8k_s = w8p.tile([128, CCH, DG], FP8, tag="w8k_s")

                # Input loads: ALL on one queue (SP): transfers from
                # different queues round-robin on the shared DMA device, so
                # a single queue is the only way to enforce global
                # dependency-criticality order. x8hi ships in column halves
                # so q/k-projection of the first two tq blocks (and the
                # first v rounds) can start before the full tensor lands.
                wvh_r = wv8hi.rearrange("(a p) b -> p a b", p=128)
                wvl_r = wv8lo.rearrange("(a p) b -> p a b", p=128)
                xhi_r = x8hi.rearrange("(a p) t -> p a t", p=128)
                xlo_r = x8lo.rearrange("(a p) t -> p a t", p=128)
                H2 = T // 2
                nc.sync.dma_start(wv8hi_s[:, 0:4, :], wvh_r[:, 0:4, :])
                nc.sync.dma_start(x8hi_s[:, 0:4, 0:H2], xhi_r[:, 0:4, 0:H2])
                nc.sync.dma_start(x8hi_s[:, 4:8, 0:H2], xhi_r[:, 4:8, 0:H2])
                nc.sync.dma_start(wv8hi_s[:, 4:16, :], wvh_r[:, 4:16, :])
                nc.sync.dma_start(x8hi_s[:, 8:16, 0:H2], xhi_r[:, 8:16, 0:H2])
                nc.sync.dma_start(
                    w8q_s[:, :, :], w8q.rearrange("(a p) b -> p a b", p=128)
                )
                nc.sync.dma_start(
                    w8k_s[:, :, :], w8k.rearrange("(a p) b -> p a b", p=128)
                )
                nc.sync.dma_start(x8hi_s[:, 0:8, H2:T], xhi_r[:, 0:8, H2:T])
                nc.sync.dma_start(x8hi_s[:, 8:16, H2:T], xhi_r[:, 8:16, H2:T])
                nc.sync.dma_start(cfq_s[:D, :], cfq)
                nc.sync.dma_start(cfk_s[:D, :], cfk)
                nc.sync.dma_start(wv8lo_s[:, :, :], wvl_r)
                nc.sync.dma_start(x8lo_s[:, 0:8, :], xlo_r[:, 0:8, :])
                nc.sync.dma_start(x8lo_s[:, 8:16, :], xlo_r[:, 8:16, :])
                nc.sync.dma_start(
                    wo8hi_s[:D, :, :], wo8hi.rearrange("h p c -> p h c")
                )
                nc.sync.dma_start(
                    wo8lo_s[:D, :, :], wo8lo.rearrange("h p c -> p h c")
                )

                # A1/A3 rounds and A2's qk tiles all rotate through one
                # 8-bank tag space -- no PSUM scope transitions in phase A.
                def alloc_accs(n):
                    accs = []
                    if n > 0:
                        w = ps_pair("w0")
                        accs += [w[:, 0, :], w[:, 1, :]]
                    if n > 2:
                        w = ps_pair("w1")
                        accs += [w[:, 0, :], w[:, 1, :]]
                    for tag in ("o2p0", "o2p1", "sp", "fp")[: max(0, n - 4)]:
                        accs.append(ps_one(tag)[:, :])
                    return accs[:n]

                def alloc_sing(n, off=0):
                    return [
                        ps_one(tag)[:, :]
                        for tag in ("o2p0", "o2p1", "sp", "fp")[off : off + n]
                    ]

                def vchain(round_tks, chains, vop, accs):
                    pvs = accs
                    n = len(chains) * CCP
                    k = 0
                    for ccp in range(CCP):
                        for x_s, w_s in chains:
                            for i, tk in enumerate(round_tks):
                                nc.tensor.matmul(
                                    pvs[i][:],
                                    x_s[:, 2 * ccp : 2 * ccp + 2, tk * 128 : (tk + 1) * 128],
                                    w_s[:, 2 * ccp : 2 * ccp + 2, :],
                                    start=(k == 0),
                                    stop=(k == n - 1),
                                    perf_mode=DR,
                                )
                            k += 1
                    for i, tk in enumerate(round_tks):
                        vop(tk, pvs[i])

                # A1: v-proj chain 1 (x_hi @ Wv_hi) -> v_sb (holds 64*v);
                # PSUM->SBUF copies alternate DVE/ACT
                def a1_copy(tk, pv):
                    if tk % 2 == 0:
                        nc.vector.tensor_copy(v_sb[:, tk, :], pv[:])
                    else:
                        nc.scalar.activation(v_sb[:, tk, :], pv[:], Copy)

                a3_add = lambda tk, pv: nc.vector.tensor_add(
                    v_sb[:, tk, :], v_sb[:, tk, :], pv[:]
                )

                # q/k tile generator: one projection output tile at a time,
                # using a caller-supplied PSUM slot
                def qk_tile(h, wi, tq, qp):
                    w_s, cf_s, dst = (
                        (w8q_s, cfq_s, qT16) if wi == 0 else (w8k_s, cfk_s, kT16)
                    )
                    for ccp in range(CCP):
                        nc.tensor.matmul(
                            qp[:],
                            w_s[:, 2 * ccp : 2 * ccp + 2, h * D : (h + 1) * D],
                            x8hi_s[:, 2 * ccp : 2 * ccp + 2, tq * 512 : (tq + 1) * 512],
                            start=(ccp == 0),
                            stop=(ccp == CCP - 1),
                            perf_mode=DR,
                        )
                    nc.vector.tensor_mul(
                        dst[:D, h, tq * 512 : (tq + 1) * 512],
                        qp[:D, :],
                        cf_s[:D, tq * 512 : (tq + 1) * 512],
                    )

                qk_left = [
                    (h, wi, tq) for h in range(HG) for wi in (0, 1) for tq in (0, 1)
                ]
                qk_right = [
                    (h, wi, tq) for h in range(HG) for wi in (0, 1) for tq in (2, 3)
                ]
                spfp = [0]

                def emit_qk(units, n):
                    # constrained 2-bank rotation (sp/fp) for tiles that
                    # interleave with open v rounds
                    for _ in range(n):
                        if not units:
                            return
                        h, wi, tq = units.pop(0)
                        qp = ps_one("sp" if spfp[0] % 2 == 0 else "fp")
                        spfp[0] += 1
                        qk_tile(h, wi, tq, qp)

                qk_accs = [None]
                qk_i = [0]

                def emit_qk_full(units, n):
                    # full 8-slot rotation once all v banks are free
                    for _ in range(n):
                        if not units:
                            return
                        h, wi, tq = units.pop(0)
                        if qk_i[0] % 8 == 0:
                            qk_accs[0] = alloc_accs(8)
                        qp = qk_accs[0][qk_i[0] % 8]
                        qk_i[0] += 1
                        qk_tile(h, wi, tq, qp)

                # r0 (tk0-5): paced by the x8hi left half
                vchain(list(range(6)), [(x8hi_s, wv8hi_s)], a1_copy, alloc_accs(6))
                # r1 (tk6-11): tk8+ paced by the right half; interleave the
                # left-half q/k tiles (ready once w8q/w8k land) on sp/fp
                pvs = alloc_accs(6)
                for ccp in range(CCP):
                    for i, tk in enumerate(range(6, 12)):
                        nc.tensor.matmul(
                            pvs[i][:],
                            x8hi_s[:, 2 * ccp : 2 * ccp + 2, tk * 128 : (tk + 1) * 128],
                            wv8hi_s[:, 2 * ccp : 2 * ccp + 2, :],
                            start=(ccp == 0),
                            stop=(ccp == CCP - 1),
                            perf_mode=DR,
                        )
                    emit_qk(qk_left, 2)
                for i, tk in enumerate(range(6, 12)):
                    a1_copy(tk, pvs[i])
                # r2 (tk12-15) + remaining left-half q/k tiles
                pvs = alloc_accs(4)
                for ccp in range(CCP):
                    for i, tk in enumerate(range(12, 16)):
                        nc.tensor.matmul(
                            pvs[i][:],
                            x8hi_s[:, 2 * ccp : 2 * ccp + 2, tk * 128 : (tk + 1) * 128],
                            wv8hi_s[:, 2 * ccp : 2 * ccp + 2, :],
                            start=(ccp == 0),
                            stop=(ccp == CCP - 1),
                            perf_mode=DR,
                        )
                    emit_qk(qk_left, 1)
                for i, tk in enumerate(range(12, 16)):
                    a1_copy(tk, pvs[i])
                emit_qk_full(qk_left, 99)
                emit_qk_full(qk_right, 99)
                _w8_cm.__exit__(None, None, None)

                # A3: v-proj correction chains (x_lo@Wv_hi + x_hi@Wv_lo);
                # later rounds avoid w0/w1 so phase B's score banks free early
                a3_chains = [(x8lo_s, wv8hi_s), (x8hi_s, wv8lo_s)]
                vchain(list(range(0, 8)), a3_chains, a3_add, alloc_accs(8))
                vchain(list(range(8, 12)), a3_chains, a3_add, alloc_accs(4))
                # tk12-15 are deferred: their 64 matmuls drip into phase B's
                # first two slots (which are PE-idle during pipeline warm-up)
                # on the single-bank tags, which phase B needs only at slot 2+
                a3_accs = alloc_sing(2) + alloc_sing(2, off=2)
                a3_tail = []
                for i, tk in enumerate(range(12, 16)):
                    for ccp in range(CCP):
                        for ci, (x_s, w_s) in enumerate(a3_chains):
                            a3_tail.append(
                                (a3_accs[i], x_s, w_s, ccp, tk,
                                 ccp == 0 and ci == 0,
                                 ccp == CCP - 1 and ci == 1)
                            )

                def drip_a3(n):
                    for _ in range(n):
                        if not a3_tail:
                            return
                        pv, x_s, w_s, ccp, tk, st, sp_ = a3_tail.pop(0)
                        nc.tensor.matmul(
                            pv[:],
                            x_s[:, 2 * ccp : 2 * ccp + 2, tk * 128 : (tk + 1) * 128],
                            w_s[:, 2 * ccp : 2 * ccp + 2, :],
                            start=st,
                            stop=sp_,
                            perf_mode=DR,
                        )
                        if sp_:
                            a3_add(tk, pv)

            # ---- phase B: slot-pipelined attention + out-projection ----
            # Slot k = (tq, h). PE emits scores(k) interleaved with
            # attn@v(k-1) -- a FULL slot of lag, so attn@v never waits on
            # ACT's exp stream. Finisher(k-1) (denominator matmul, recip,
            # normalize) and out-proj drips fill the remaining PE slack.
            if True:
                og_queue = []
                og_i = [0]
                ot4 = [None]
                fin_queue = []
                TAGROT = ["fp", "sp"]

                def emit_og(tags=("fp", "sp")):
                    tag = tags[og_i[0] % len(tags)]
                    og_i[0] += 1
                    qt, oc, o2t, tq = og_queue.pop(0)
                    fp = (
                        ps_one(tag) if tag in ("fp", "sp", "o2p0", "o2p1")
                        else ps_pair(tag[:2])[:, int(tag[3]), :]
                    )
                    o2h, o2l = o2t
                    chains = ((o2h, wo8hi_s), (o2l, wo8hi_s), (o2h, wo8lo_s))
                    n = len(chains) * 2
                    ci = 0
                    for lhs, rhs in chains:
                        for hp in (0, 2):
                            nc.tensor.matmul(
                                fp[:],
                                lhs[:D, hp : hp + 2, qt * 128 : (qt + 1) * 128],
                                rhs[:D, hp : hp + 2, oc * 512 : (oc + 1) * 512],
                                start=(ci == 0),
                                stop=(ci == n - 1),
                                perf_mode=DR,
                            )
                            ci += 1
                    if oc == 0:
                        ot4[0] = otp.tile([128, 4, 512], FP16, tag="ot", name="ot")
                    ot = ot4[0]
                    if og_i[0] % 2 == 0:
                        nc.scalar.activation(
                            ot[:, oc, :],
                            fp[:],
                            Copy,
                            scale=float(1.0 / (WSCALE * O2SCALE)),
                        )
                    else:
                        nc.vector.tensor_scalar_mul(
                            ot[:, oc, :], fp[:], 1.0 / (WSCALE * O2SCALE)
                        )
                    if oc == 3:
                        nc.sync.dma_start(
                            out[tq * 512 + qt * 128 : tq * 512 + (qt + 1) * 128, :],
                            ot[:],
                        )

                def flush_fin(tag="sp"):
                    # out-proj groups for a tq block enter the queue HERE,
                    # once its last head is normalized -- dripping them any
                    # earlier would read o2t[h3] before it is written.
                    acc, o2p, o2t, h, tq = fin_queue.pop(0)
                    spp = ps_one(tag)
                    nc.tensor.matmul(spp[:], ones[:], acc[:], start=True, stop=True)
                    rt = rtp.tile([128, 512], F32, tag="rt", bufs=1)
                    nc.vector.reciprocal(rt[:], spp[:])
                    m32 = rtp.tile([128, 512], FP16, tag="m32", bufs=2)
                    nc.vector.tensor_mul(m32[:D, :], o2p[:D, :], rt[:D, :])
                    o2h, o2l = o2t
                    nc.vector.tensor_copy(o2h[:D, h, :], m32[:D, :])
                    nc.vector.tensor_tensor(o2l[:D, h, :], m32[:D, :], o2h[:D, h, :], mybir.AluOpType.subtract)
                    if h == HG - 1:
                        for qt in range(4):
                            for oc in range(4):
                                og_queue.append((qt, oc, o2t, tq))

                slots = [(tq, h) for tq in range(NTQ) for h in range(HG)]
                pend = []  # (ctx, g) attn@v work lagging 4 exp groups

                def emit_av(ctx, g):
                    ets, o2p, o2t, h, acc, _tq = ctx
                    et = ets[g]
                    for j in range(2):
                        tk = 2 * g + j
                        nc.tensor.matmul(
                            o2p[:],
                            v_sb[:, tk, h * D : (h + 1) * D],
                            et[:, j, :],
                            start=(tk == 0),
                            stop=(tk == NTK - 1),
                        )
                    eng = nc.vector if g % 2 == 0 else nc.gpsimd
                    eng.tensor_add(et[:, 0, :], et[:, 0, :], et[:, 1, :])
                    if g == 1:
                        nc.vector.tensor_add(acc[:], ets[0][:, 0, :], ets[1][:, 0, :])
                    elif g > 1:
                        nc.vector.tensor_add(acc[:], acc[:], et[:, 0, :])

                for k, (tq, h) in enumerate(slots):
                    if h == 0:
                        o2t = (
                            o2sb.tile([128, HG, 512], FP8, tag="o2h", name="o2h"),
                            o2sb.tile([128, HG, 512], FP8, tag="o2l", name="o2l"),
                        )
                    o2p = ps_one(f"o2p{k % 2}")
                    acc = accp.tile([128, 512], FP16, tag="acc")
                    ets = []
                    ctx = (ets, o2p, o2t, h, acc, tq)
                    for g in range(NG):
                        sc = ps_pair(f"w{g % 2}")
                        for j in range(2):
                            tk = 2 * g + j
                            nc.tensor.matmul(
                                sc[:, j, :],
                                kT16[:D, h, tk * 128 : (tk + 1) * 128],
                                qT16[:D, h, tq * 512 : (tq + 1) * 512],
                                start=True,
                                stop=True,
                            )
                        if use_mask:
                            mt = mkp.tile([128, 2, 512], F32, tag="mt")
                            for j in range(2):
                                nc.gpsimd.dma_start(
                                    mt[:, j, :],
                                    maskT[
                                        (2 * g + j) * 128 : (2 * g + j + 1) * 128,
                                        tq * 512 : (tq + 1) * 512,
                                    ],
                                )
                            nc.vector.tensor_add(sc[:, :, :], sc[:, :, :], mt[:])
                        et = etp.tile(
                            [128, 2, 512], FP16, tag=f"et{(k * NG + g) % 8}"
                        )
                        nc.scalar.activation(et[:], sc[:], Exp)
                        ets.append(et)
                        drip_a3(4)
                        # attn@v lags the exp stream by 4 groups
                        pend.append((ctx, g))
                        if len(pend) > 4:
                            actx, ag = pend.pop(0)
                            emit_av(actx, ag)
                            if ag == NG - 1:
                                fin_queue.append(
                                    (actx[4], actx[1], actx[2], actx[3], actx[5])
                                )
                        if g == 4 and fin_queue:
                            flush_fin()
                        if g in (3, 5, 7) and og_queue:
                            emit_og()
                            if g == 7 and og_queue and len(og_queue) % 4 == 1:
                                emit_og()
                _xw_cm.__exit__(None, None, None)
                # drain: pending attn@v groups, remaining finishers, then the
                # final out-proj groups rotating over the idle banks
                di = 0
                while pend:
                    actx, ag = pend.pop(0)
                    emit_av(actx, ag)
                    if ag == NG - 1:
                        fin_queue.append(
                            (actx[4], actx[1], actx[2], actx[3], actx[5])
                        )
                    if di % 2 == 1 and og_queue:
                        emit_og()
                    di += 1
                while fin_queue:
                    flush_fin("sp" if len(fin_queue) % 2 == 0 else "fp")
                og_i[0] = 0
                while og_queue:
                    emit_og(
                        ("fp", "w0_0", "sp", "w1_0", "o2p0", "w0_1", "o2p1", "w1_1")
                    )

            _ps_cm.__exit__(None, None, None)

    nc.compile()
    return nc


def compute_cfacs():
    T, D = SEQ, HEAD_DIM
    freq = 1.0 / THETA ** (np.arange(0, D, 2, dtype=np.float64) / D)
    t = np.arange(T, dtype=np.float64)
    m = np.einsum("i,j->ij", t, freq)
    m = np.concatenate([m, m], axis=-1)
    cfac = (np.cos(m) + np.sin(m)).T  # [D, T]
    cfq = (cfac / math.sqrt(T) / WSCALE).astype(np.float16)
    cfk = (cfac / WSCALE).astype(np.float16)
    return cfq, cfk


_NC_CACHE = {}


def _get_nc(use_mask):
    key = bool(use_mask)
    if key not in _NC_CACHE:
        _NC_CACHE[key] = build_attention_nc(use_mask=key)
    return _NC_CACHE[key]


def _split_fp8(a):
    hi = a.astype(NP_FP8)
    lo = (a - hi.astype(np.float32)).astype(NP_FP8)
    return hi, lo


def _make_in_maps(input_ids, Wq, Wk, Wv, Wo, attention_mask=None):
    DG = HG * HEAD_DIM
    cfq, cfk = compute_cfacs()
    xhi, xlo = [], []
    for bi in range(BATCH):
        xT = np.ascontiguousarray(input_ids[bi].T)
        hi, lo = _split_fp8(xT)
        xhi.append(hi)
        xlo.append(lo)
    in_maps = []
    for core in range(N_CORES):
        bi, g = divmod(core, MP)
        sl = slice(g * DG, (g + 1) * DG)
        wvhi, wvlo = _split_fp8(np.ascontiguousarray(Wv[:, sl]) * WSCALE)
        wohi, wolo = _split_fp8(
            np.ascontiguousarray(Wo[sl, :]).reshape(HG, HEAD_DIM, HIDDEN) * WSCALE
        )
        m = {
            "x8hi": xhi[bi],
            "x8lo": xlo[bi],
            "wv8hi": wvhi,
            "wv8lo": wvlo,
            "w8q": (np.ascontiguousarray(Wq[:, sl]) * WSCALE).astype(NP_FP8),
            "w8k": (np.ascontiguousarray(Wk[:, sl]) * WSCALE).astype(NP_FP8),
            "wo8hi": wohi,
            "wo8lo": wolo,
            "cfq": cfq,
            "cfk": cfk,
        }
        if attention_mask is not None:
            m["maskT"] = np.ascontiguousarray(attention_mask[bi, 0].T).astype(
                np.float32
            )
        in_maps.append(m)
    return in_maps


def prepare_for_bench(inputs):
    input_ids = np.asarray(inputs["input_ids"], dtype=np.float32)
    Wq = np.asarray(inputs["Wq"], dtype=np.float32)
    Wk = np.asarray(inputs["Wk"], dtype=np.float32)
    Wv = np.asarray(inputs["Wv"], dtype=np.float32)
    Wo = np.asarray(inputs["Wo"], dtype=np.float32)
    return _get_nc(False), _make_in_maps(input_ids, Wq, Wk, Wv, Wo)


def kernel(input_ids, attention_mask, Wq, Wk, Wv, Wo):
    input_ids = np.asarray(input_ids, dtype=np.float32)
    attention_mask = np.asarray(attention_mask, dtype=np.float32)
    Wq = np.asarray(Wq, dtype=np.float32)
    Wk = np.asarray(Wk, dtype=np.float32)
    Wv = np.asarray(Wv, dtype=np.float32)
    Wo = np.asarray(Wo, dtype=np.float32)

    b, t, c = input_ids.shape
    assert (b, t, c) == (BATCH, SEQ, HIDDEN)

    use_mask = bool(np.any(attention_mask))
    nc = _get_nc(use_mask)
    in_maps = _make_in_maps(
        input_ids, Wq, Wk, Wv, Wo, attention_mask if use_mask else None
    )

    res = bass_utils.run_bass_kernel_spmd(nc, in_maps, core_ids=list(range(N_CORES)))

    out = np.zeros((BATCH, SEQ, HIDDEN), dtype=np.float32)
    for bi in range(BATCH):
        acc = res.results[bi * MP]["out"].astype(np.float32)
        for g in range(1, MP):
            acc = acc + res.results[bi * MP + g]["out"].astype(np.float32)
        out[bi] = acc
    return out
